# revision 14
# baseline (speedup 1.0000x reference)
"""Trainium2 Bass kernel for nn_MeshConv (ChebConv K=2, two layers) on 8 cores.

Math (reference):
    deg  = bincount(src)                          # out-degree over src column
    dinv = where(deg>0, rsqrt(max(deg,1)), 0)
    z    = segment_sum(-dinv[src]*dinv[dst] * x[src], dst)
         = -dinv[dst] * segment_sum((dinv*x)[src], dst)
    layer(x) = x @ W0 + z-term @ W1 + b     (layer1 wrapped in relu)

Device strategy (per core, dst-sharded; two dispatches with a host hop):
  * LANE-PACKED gather tables: each 256B row holds TWO nodes' payloads
    (64 bf16 each). The host pairs nodes that share a (dst-block, chunk)
    cell, so one fetch often serves two edges; per-fetch dual one-hots
    (slotA/slotB, 255 = no edge) feed two PE matmuls per tile.
  * layer 1 builds its table on device from a per-core COLUMN-PERMUTED,
    dinv-prescaled vt (partition-major staging -> one linear full-BW DMA
    per chunk); layer 2's table is packed directly on the host from the
    layer-1 output, with W1_2 applied AFTER the segment-sum (it commutes).
  * dst nodes re-binned into NB=110 balanced blocks (LPT) so the
    per-(block,chunk) fetch cap is T_BQ=3 tiles.
  * one-hot S built on DVE in [edge, slot, tile] layout (all APs packed in
    the last dim -> 2x 16-bit DVE mode); epilogue relu/copies on the
    Activation engine; dense term via augmented (ones/bias) matrix.
"""
import os
import numpy as np
import ml_dtypes

import concourse.bacc as bacc
import concourse.tile as tile
import concourse.mybir as mybir
import concourse.bass as bass
from concourse import library_config
from concourse.bass_utils import run_bass_kernel_spmd

P = 128
NOSLOT = 255.0            # slot value that never matches iota 0..127

# exported for test.py: exec times of the two dispatches when tracing
LAST_EXEC_NS = []


class Cfg:
    def __init__(self, n_nodes, n_edges, n_cores, in_dim, h1, h2):
        self.N, self.E, self.C = n_nodes, n_edges, n_cores
        self.IN, self.H1, self.H2 = in_dim, h1, h2
        self.OWN = n_nodes // n_cores                 # owned dst nodes / core
        self.NB = 112                                 # balanced dst blocks
        assert self.NB * P >= self.OWN
        self.NODES_PAD = self.NB * P
        # src chunking for int16 gather indices
        self.NCH = 4
        self.CHUNK_REAL = -(-n_nodes // self.NCH)
        self.CHUNK_REAL = -(-self.CHUNK_REAL // P) * P   # 25088
        self.NT_CH = self.CHUNK_REAL // P                # 196 build tiles
        self.NTH = self.NT_CH // 2                       # 98 packed rows/stripe
        self.TROWS = self.NTH                            # rows per partition
        assert P * self.TROWS <= 32767
        self.NPADCOL = self.NCH * self.CHUNK_REAL
        # gather call grouping
        self.GRP = 7
        self.GROUPS = []
        b = 0
        while b < self.NB:
            n = min(self.GRP, self.NB - b)
            self.GROUPS.append((b, n))
            b += n

    def set_tbq(self, t_bq):
        self.T_BQ = t_bq
        self.CAP = t_bq * P
        self.TOTAL = self.NB * self.NCH * self.CAP       # fetch slots / layer
        self.TTOT = self.TOTAL // P


def _gather_phase(nc, tc, c, tables, idx, slotA, slotB, iota_t, dt,
                  emit_matmuls, emit_epilogue):
    """Shared gather + dual-one-hot + segment-matmul + epilogue loop."""
    TGF = c.GRP * c.T_BQ
    tbl_flat = [tables[q][:, :, :].rearrange("p t c -> (p t) c")
                for q in range(c.NCH)]
    with tc.tile_pool(name="gat", bufs=2) as gpool, \
         tc.tile_pool(name="epi", bufs=2) as epool, \
         tc.tile_pool(name="gpsum", bufs=7, space="PSUM") as gpsum, \
         tc.tile_pool(name="zpsum", bufs=1, space="PSUM") as zpsum:
        goff = 0
        for (b0, nblk) in c.GROUPS:
            nidx = nblk * c.CAP
            tg = nidx // P
            idx_t = gpool.tile([P, c.NCH * c.GRP * c.CAP // 16],
                               dt.int16, tag="ix")
            nc.sync.dma_start(
                idx_t[:, 0:c.NCH * nidx // 16],
                idx[:, goff // 16:(goff + c.NCH * nidx) // 16])
            slA_t = gpool.tile([P, c.NCH * c.GRP * c.T_BQ], dt.bfloat16,
                               tag="slA")
            nc.sync.dma_start(
                slA_t[:, 0:c.NCH * tg],
                slotA[:, goff // P:(goff + c.NCH * nidx) // P])
            slB_t = gpool.tile([P, c.NCH * c.GRP * c.T_BQ], dt.bfloat16,
                               tag="slB")
            nc.sync.dma_start(
                slB_t[:, 0:c.NCH * tg],
                slotB[:, goff // P:(goff + c.NCH * nidx) // P])
            zts = [gpsum.tile(emit_matmuls.zshape, dt.float32, space="PSUM",
                              tag="zt", name="zt") for _ in range(nblk)]
            for q in range(c.NCH):
                g_t = gpool.tile([P, TGF, P], dt.bfloat16, tag="g")
                SUB = 1024
                for sb in range(0, nidx, SUB):
                    sn = min(SUB, nidx - sb)
                    i0 = q * nidx + sb
                    nc.gpsimd.dma_gather(
                        g_t[:, sb // P:(sb + sn) // P, :], tbl_flat[q],
                        idx_t[:, i0 // 16:(i0 + sn) // 16], sn, sn, P)
                s_ohs = []
                for (lane, sl_t) in ((0, slA_t), (1, slB_t)):
                    s_oh = gpool.tile([P, P, TGF], dt.bfloat16,
                                      tag=f"s{lane}")
                    nc.vector.tensor_tensor(
                        out=s_oh[:, :, 0:tg],
                        in0=sl_t[:, q * tg:(q + 1) * tg]
                            .unsqueeze(1).to_broadcast([P, P, tg]),
                        in1=iota_t[:, :, 0:tg],
                        op=mybir.AluOpType.is_equal)
                    s_ohs.append(s_oh)
                for br in range(nblk):
                    for tr in range(c.T_BQ):
                        tt = br * c.T_BQ + tr
                        for lane in range(2):
                            emit_matmuls(
                                zts[br], g_t, s_ohs[lane], tt, lane,
                                start=(q == 0 and tr == 0 and lane == 0),
                                stop=(q == c.NCH - 1 and
                                      tr == c.T_BQ - 1 and lane == 1))
            emit_epilogue(epool, zpsum, zts, b0, nblk)
            goff += c.NCH * nidx


def _build_layer1(cfg, kf, m_out):
    """Layer-1 dispatch: on-device lane-packed table build + dense +
    gather/segment-matmul (node-major z) + relu epilogue."""
    c = cfg
    nc = bacc.Bacc("TRN2", target_bir_lowering=False, debug=False)
    dt = mybir.dt

    vt = nc.dram_tensor("vt", [kf, c.NPADCOL], dt.bfloat16,
                        kind="ExternalInput")
    vox = nc.dram_tensor("vox", [kf + 1, c.NODES_PAD], dt.bfloat16,
                         kind="ExternalInput")
    w_t = nc.dram_tensor("w_t", [kf, m_out], dt.bfloat16,
                         kind="ExternalInput")
    w_d = nc.dram_tensor("w_d", [kf + 1, m_out], dt.bfloat16,
                         kind="ExternalInput")
    dinv_n = nc.dram_tensor("dinv_n", [P, c.NB], dt.float32,
                            kind="ExternalInput")
    idx = nc.dram_tensor("idx", [P, c.TOTAL // 16], dt.int16,
                         kind="ExternalInput")
    slotA = nc.dram_tensor("slotA", [P, c.TTOT], dt.bfloat16,
                           kind="ExternalInput")
    slotB = nc.dram_tensor("slotB", [P, c.TTOT], dt.bfloat16,
                           kind="ExternalInput")
    TGF = c.GRP * c.T_BQ
    iota = nc.dram_tensor("iota", [P, P, TGF], dt.bfloat16,
                          kind="ExternalInput")
    out = nc.dram_tensor("out", [P, c.NB, m_out], dt.bfloat16,
                         kind="ExternalOutput")
    tables = [
        nc.dram_tensor(f"table{q}", [P, c.TROWS, P], dt.bfloat16)
        for q in range(c.NCH)
    ]

    def chunks(k):
        r, out_ = 0, []
        while r < k:
            n = min(P, k - r)
            out_.append((r, n))
            r += n
        return out_

    kchunks = chunks(kf)
    dchunks = chunks(kf + 1)
    TB = 28                         # build tiles per vt load (196 = 7*28)
    HB = 14                         # psum batch: 7 packed rows

    with tile.TileContext(nc) as tc:
        with tc.tile_pool(name="const", bufs=1) as cpool:
            nc.gpsimd.load_library(library_config.mlp)

            iota_t = cpool.tile([P, P, TGF], dt.bfloat16)
            nc.sync.dma_start(iota_t[:], iota[:, :, :])
            dinv_t = cpool.tile([P, c.NB], dt.float32)
            nc.sync.dma_start(dinv_t[:], dinv_n[:, :])
            wt_t = [cpool.tile([n, m_out], dt.bfloat16, tag=f"wt{i}",
                               name=f"wt{i}")
                    for i, (r0, n) in enumerate(kchunks)]
            wd_t = [cpool.tile([n, m_out], dt.bfloat16, tag=f"wd{i}",
                               name=f"wd{i}")
                    for i, (r0, n) in enumerate(dchunks)]
            for i, (r0, n) in enumerate(kchunks):
                nc.sync.dma_start(wt_t[i][:], w_t[r0:r0 + n, :])
            for i, (r0, n) in enumerate(dchunks):
                nc.sync.dma_start(wd_t[i][:], w_d[r0:r0 + n, :])
            dense_t = cpool.tile([P, c.NB, m_out], dt.bfloat16)

            # ---- table build: tile t -> stg[:, t//2, (t%2)*64:...] ----
            with tc.tile_pool(name="stgp", bufs=2) as spool, \
                 tc.tile_pool(name="bld", bufs=3) as bpool, \
                 tc.tile_pool(name="bpsum", bufs=3, space="PSUM") as bpsum:
                for q in range(c.NCH):
                    stg = spool.tile([P, c.NTH, P], dt.bfloat16, tag="stg")
                    for t0 in range(0, c.NT_CH, TB):
                        j0 = (q * c.NT_CH + t0) * P
                        vtiles = [bpool.tile([n, TB * P], dt.bfloat16,
                                             tag=f"v{i}", name=f"v{i}")
                                  for i, (r0, n) in enumerate(kchunks)]
                        for i, (r0, n) in enumerate(kchunks):
                            nc.sync.dma_start(
                                vtiles[i][:], vt[r0:r0 + n, j0:j0 + TB * P])
                        for h0 in range(0, TB, HB):
                            ps = bpsum.tile([P, HB // 2, P], dt.float32,
                                            tag="bps", space="PSUM")
                            for k in range(HB):
                                lane = k % 2
                                for i, (r0, n) in enumerate(kchunks):
                                    nc.tensor.matmul(
                                        out=ps[:, k // 2,
                                               lane * m_out:(lane + 1) * m_out],
                                        lhsT=vtiles[i][:, (h0 + k) * P:
                                                       (h0 + k + 1) * P],
                                        rhs=wt_t[i][:], start=(i == 0),
                                        stop=(i == len(kchunks) - 1))
                            th0 = (t0 + h0) // 2
                            nc.scalar.activation(
                                out=stg[:, th0:th0 + HB // 2, :], in_=ps[:],
                                func=mybir.ActivationFunctionType.Copy)
                    nc.sync.dma_start(tables[q][:, :, :], stg[:])

                # ---- dense term: node-major x@W0 + bias (augmented) ----
                for j0 in range(0, c.NB, HB):
                    nj = min(HB, c.NB - j0)
                    voxt = [bpool.tile([n, HB * P], dt.bfloat16,
                                       tag=f"x{i}", name=f"x{i}")
                            for i, (r0, n) in enumerate(dchunks)]
                    for i, (r0, n) in enumerate(dchunks):
                        nc.sync.dma_start(
                            voxt[i][:, 0:nj * P],
                            vox[r0:r0 + n, j0 * P:(j0 + nj) * P])
                    ps = bpsum.tile([P, HB // 2, P], dt.float32, tag="bps",
                                    space="PSUM")
                    for k in range(nj):
                        lane = k % 2
                        for i, (r0, n) in enumerate(dchunks):
                            nc.tensor.matmul(
                                out=ps[:, k // 2,
                                       lane * m_out:(lane + 1) * m_out],
                                lhsT=voxt[i][:, k * P:(k + 1) * P],
                                rhs=wd_t[i][:], start=(i == 0),
                                stop=(i == len(dchunks) - 1))
                    nc.scalar.activation(
                        out=dense_t[:, j0:j0 + nj, :]
                        .rearrange("p a b -> p (a b)"),
                        in_=ps[:].rearrange("p a b -> p (a b)")
                        [:, 0:nj * m_out],
                        func=mybir.ActivationFunctionType.Copy)

            # ---- gather + dual one-hot + epilogue ----
            def emit_mm(zt, g_t, s_oh, tt, lane, start, stop):
                nc.tensor.matmul(
                    out=zt[:], lhsT=s_oh[:, :, tt],
                    rhs=g_t[:, tt, lane * m_out:(lane + 1) * m_out],
                    start=start, stop=stop)
            emit_mm.zshape = [P, m_out]

            def emit_epi(epool, zpsum, zts, b0, nblk):
                ob = epool.tile([P, c.GRP, m_out], dt.bfloat16, tag="ob")
                for br in range(nblk):
                    blk = b0 + br
                    tmp = epool.tile([P, m_out], dt.float32, tag="tmp")
                    nc.vector.tensor_scalar(
                        out=tmp[:], in0=zts[br][:],
                        scalar1=dinv_t[:, blk:blk + 1], scalar2=None,
                        op0=mybir.AluOpType.mult)
                    nc.vector.tensor_tensor(
                        out=tmp[:], in0=dense_t[:, blk, :], in1=tmp[:],
                        op=mybir.AluOpType.subtract)
                    nc.scalar.activation(
                        out=ob[:, br, :], in_=tmp[:],
                        func=mybir.ActivationFunctionType.Relu)
                nc.sync.dma_start(out[:, b0:b0 + nblk, :], ob[:, 0:nblk, :])

            _gather_phase(nc, tc, c, tables, idx, slotA, slotB, iota_t, dt,
                          emit_mm, emit_epi)
    nc.compile()
    return nc


def _build_layer2(cfg, kf, m_out):
    """Layer-2 dispatch: HOST-BUILT lane-packed table of (dinv*h); z
    accumulated in h-space [kf, slots]; W1_2 applied post-sum; feature-major
    dense/epilogue."""
    c = cfg
    nc = bacc.Bacc("TRN2", target_bir_lowering=False, debug=False)
    dt = mybir.dt

    vox = nc.dram_tensor("vox", [kf + 1, c.NODES_PAD], dt.bfloat16,
                         kind="ExternalInput")
    w_t = nc.dram_tensor("w_t", [kf, m_out], dt.bfloat16,
                         kind="ExternalInput")
    w_d = nc.dram_tensor("w_d", [kf + 1, m_out], dt.bfloat16,
                         kind="ExternalInput")
    dinv_r = nc.dram_tensor("dinv_r", [m_out, c.NODES_PAD], dt.bfloat16,
                            kind="ExternalInput")
    idx = nc.dram_tensor("idx", [P, c.TOTAL // 16], dt.int16,
                         kind="ExternalInput")
    slotA = nc.dram_tensor("slotA", [P, c.TTOT], dt.bfloat16,
                           kind="ExternalInput")
    slotB = nc.dram_tensor("slotB", [P, c.TTOT], dt.bfloat16,
                           kind="ExternalInput")
    TGF = c.GRP * c.T_BQ
    iota = nc.dram_tensor("iota", [P, P, TGF], dt.bfloat16,
                          kind="ExternalInput")
    tables = [
        nc.dram_tensor(f"table{q}", [P, c.TROWS, P], dt.bfloat16,
                       kind="ExternalInput")
        for q in range(c.NCH)
    ]
    out = nc.dram_tensor("out", [m_out, c.NODES_PAD], dt.float32,
                         kind="ExternalOutput")

    with tile.TileContext(nc) as tc:
        with tc.tile_pool(name="const", bufs=1) as cpool:
            nc.gpsimd.load_library(library_config.mlp)

            iota_t = cpool.tile([P, P, TGF], dt.bfloat16)
            nc.sync.dma_start(iota_t[:], iota[:, :, :])
            wt_t = cpool.tile([kf, m_out], dt.bfloat16)
            nc.sync.dma_start(wt_t[:], w_t[:, :])
            wd_t = cpool.tile([kf + 1, m_out], dt.bfloat16)
            nc.sync.dma_start(wd_t[:], w_d[:, :])
            dinv_t = cpool.tile([m_out, c.NODES_PAD], dt.bfloat16)
            nc.sync.dma_start(dinv_t[:], dinv_r[:, :])
            dense_t = cpool.tile([m_out, c.NODES_PAD], dt.bfloat16)

            # ---- dense term, feature-major, weights stationary ----
            DJ = 512
            with tc.tile_pool(name="dns", bufs=2) as dpool, \
                 tc.tile_pool(name="dpsum", bufs=2, space="PSUM") as dpsum:
                voxt = dpool.tile([kf + 1, c.NODES_PAD], dt.bfloat16,
                                  tag="vx")
                nc.sync.dma_start(voxt[:], vox[:, :])
                for j in range(0, c.NODES_PAD, DJ):
                    dj = min(DJ, c.NODES_PAD - j)
                    ps = dpsum.tile([m_out, DJ], dt.float32, tag="dps",
                                    space="PSUM")
                    nc.tensor.matmul(out=ps[:, 0:dj], lhsT=wd_t[:],
                                     rhs=voxt[:, j:j + dj],
                                     start=True, stop=True)
                    nc.scalar.activation(
                        out=dense_t[:, j:j + dj], in_=ps[:, 0:dj],
                        func=mybir.ActivationFunctionType.Copy)

            def emit_mm(zt, g_t, s_oh, tt, lane, start, stop):
                nc.tensor.matmul(
                    out=zt[:], lhsT=g_t[:, tt, lane * 64:lane * 64 + kf],
                    rhs=s_oh[:, :, tt], start=start, stop=stop)
            emit_mm.zshape = [kf, P]

            def emit_epi(epool, zpsum, zts, b0, nblk):
                ob = epool.tile([m_out, c.GRP * P], dt.float32, tag="ob")
                for br in range(nblk):
                    js = slice((b0 + br) * P, (b0 + br + 1) * P)
                    zraw = epool.tile([kf, P], dt.bfloat16, tag="zr")
                    nc.scalar.activation(
                        out=zraw[:], in_=zts[br][:],
                        func=mybir.ActivationFunctionType.Copy)
                    z2 = zpsum.tile([m_out, P], dt.float32, tag="z2",
                                    space="PSUM")
                    nc.tensor.matmul(out=z2[:], lhsT=wt_t[:], rhs=zraw[:],
                                     start=True, stop=True)
                    tmp = epool.tile([m_out, P], dt.float32, tag="tmp")
                    nc.vector.tensor_tensor(
                        out=tmp[:], in0=z2[:], in1=dinv_t[:, js],
                        op=mybir.AluOpType.mult)
                    nc.vector.tensor_tensor(
                        out=ob[:, br * P:(br + 1) * P],
                        in0=dense_t[:, js], in1=tmp[:],
                        op=mybir.AluOpType.subtract)
                nc.sync.dma_start(out[:, b0 * P:(b0 + nblk) * P],
                                  ob[:, 0:nblk * P])

            _gather_phase(nc, tc, c, tables, idx, slotA, slotB, iota_t, dt,
                          emit_mm, emit_epi)
    nc.compile()
    return nc


def _balance(cnt_vq, nb, cap=P):
    """LPT assignment of dst nodes to blocks minimizing max (block,chunk)
    load. Returns block_of[v], slot_of[v]."""
    own, nch = cnt_vq.shape
    order = np.argsort(-cnt_vq.sum(1), kind="stable")
    load = np.zeros((nb, nch), np.int64)
    bcnt = np.zeros(nb, np.int64)
    block_of = np.empty(own, np.int64)
    slot_of = np.empty(own, np.int64)
    big = 1 << 40
    for v in order:
        cv = cnt_vq[v]
        scores = (load + cv).max(axis=1) * 4096 + load.sum(axis=1) // 64
        scores = scores + big * (bcnt >= cap)
        b = int(np.argmin(scores))
        block_of[v] = b
        slot_of[v] = bcnt[b]
        load[b] += cv
        bcnt[b] += 1
    return block_of, slot_of, int(load.max())


def _pair_chunk(cfg, b_ed, r_ed, cap):
    """Pair src rows sharing a (block) cell within one chunk so per-cell
    fetch counts fit cap. Returns rowmap[r], lanemap[r], perm (node at each
    pi-position), and per-edge fetch info computed later."""
    c = cfg
    CH = c.CHUNK_REAL
    ncell = np.bincount(b_ed, minlength=c.NB)
    # per-cell unique rows (and counts) via sorted edges
    order = np.lexsort((r_ed, b_ed))
    bs, rs = b_ed[order], r_ed[order]
    newgrp = np.ones(len(bs), bool)
    if len(bs) > 1:
        newgrp[1:] = (bs[1:] != bs[:-1]) | (rs[1:] != rs[:-1])
    gb, gr = bs[newgrp], rs[newgrp]            # unique (cell,row) pairs
    gcnt = np.diff(np.flatnonzero(np.append(newgrp, True)))
    match = np.full(CH, -1, np.int64)
    cell_order = np.argsort(-ncell)
    gstart = np.zeros(c.NB + 1, np.int64)
    np.add.at(gstart, gb + 1, 1)
    gstart = np.cumsum(gstart)
    # round-robin pairing: one pair per needy cell per round so that the
    # shared free-node supply is spread evenly across cells
    need = (ncell - cap + 8).clip(min=0).astype(np.int64)
    cand = {}
    ptr = {}
    for b in np.flatnonzero(need > 0):
        rows_b = gr[gstart[b]:gstart[b + 1]]
        cnt_b = gcnt[gstart[b]:gstart[b + 1]]
        o = np.argsort(-cnt_b, kind="stable")
        cand[b] = (rows_b[o], cnt_b[o])
        ptr[b] = 0
    active = sorted(cand.keys(), key=lambda b: -need[b])
    while active:
        nxt = []
        for b in active:
            rows_b, cnt_b = cand[b]
            i = ptr[b]
            got = None
            while i < len(rows_b) - 1:
                while i < len(rows_b) and match[rows_b[i]] >= 0:
                    i += 1
                j = i + 1
                while j < len(rows_b) and match[rows_b[j]] >= 0:
                    j += 1
                if j >= len(rows_b):
                    i = len(rows_b)
                    break
                u, w = int(rows_b[i]), int(rows_b[j])
                match[u] = w
                match[w] = u
                got = min(int(cnt_b[i]), int(cnt_b[j]))
                i = j + 1
                break
            ptr[b] = i
            if got is None:
                continue                      # supply exhausted for b
            need[b] -= got
            if need[b] > 0 and i < len(rows_b) - 1:
                nxt.append(b)
        active = nxt
    # build row layout: pairs first, then remaining nodes two per row
    a_nodes = np.flatnonzero((match >= 0) & (np.arange(CH) < match))
    b_nodes = match[a_nodes]
    rest = np.flatnonzero(match < 0)
    half = (len(rest) + 1) // 2
    laneA = np.concatenate([a_nodes, rest[:half]])
    laneB_src = rest[half:]
    # rows: k-th row holds (laneA[k], laneB[k] or none)
    nrows = len(laneA)
    assert nrows <= P * c.NTH
    rowmap = np.full(CH, 0, np.int64)
    lanemap = np.zeros(CH, np.int64)
    # row k -> (p = k % P, th = k // P) -> packed row id p*NTH + th
    k = np.arange(nrows)
    rid = (k % P) * c.NTH + k // P
    rowmap[laneA] = rid
    lanemap[laneA] = 0
    kb = np.arange(len(b_nodes))
    rowmap[b_nodes] = rid[kb]
    lanemap[b_nodes] = 1
    if len(laneB_src):
        kb2 = np.arange(len(a_nodes), len(a_nodes) + len(laneB_src))
        ridb = (kb2 % P) * c.NTH + kb2 // P
        rowmap[laneB_src] = ridb
        lanemap[laneB_src] = 1
    # perm: node at pi-position u = t*128 + p where t = 2*th + lane
    perm = np.zeros(c.NT_CH * P, np.int64)
    filled = np.zeros(c.NT_CH * P, bool)
    allr = np.arange(CH)
    t_of = 2 * (rowmap[allr] // c.NTH * 0 + (rowmap[allr] % c.NTH)) \
        + lanemap[allr]
    p_of = rowmap[allr] // c.NTH
    upos = t_of * P + p_of
    perm[upos] = allr
    filled[upos] = True
    # unfilled positions (when rest is odd / rows < capacity): point at row 0
    perm[~filled] = 0
    return rowmap, lanemap, perm


def _fetch_streams(cfg, b_e, q_e, j_e, l_e, s_e):
    """Build per-cell fetch instances and fill idx/slotA/slotB streams.
    Returns (idx_arr, slA_arr, slB_arr, max_cell_fetches)."""
    c = cfg
    E = len(b_e)
    # rank edges within (cell, chunk, row, lane)
    order = np.lexsort((l_e, j_e, b_e, q_e))
    qs, bs, js, ls, ss = (q_e[order], b_e[order], j_e[order], l_e[order],
                          s_e[order])
    key_row = ((qs * c.NB + bs) * (P * c.NTH + 1) + js)
    newrow = np.ones(E, bool)
    newrow[1:] = key_row[1:] != key_row[:-1]
    key_lane = key_row * 2 + ls
    newlane = np.ones(E, bool)
    newlane[1:] = key_lane[1:] != key_lane[:-1]
    lane_start = np.maximum.accumulate(np.where(newlane, np.arange(E), 0))
    rank = np.arange(E) - lane_start                 # rank within lane run
    # fetch count per (cell,row) = max(lane ranks)+1
    row_grp = np.cumsum(newrow) - 1                  # group id per edge
    nfetch_row = np.zeros(row_grp[-1] + 1 if E else 0, np.int64)
    np.maximum.at(nfetch_row, row_grp, rank + 1)
    # per (q,b) cell fetch totals
    cellid = qs * c.NB + bs
    cell_of_row = np.zeros(len(nfetch_row), np.int64)
    cell_of_row[row_grp[newrow.nonzero()[0]] if E else []] = \
        cellid[newrow] if E else []
    cnt = np.zeros((c.NB, c.NCH), np.int64)
    np.add.at(cnt, (cell_of_row % (c.NB * c.NCH) // 1 % c.NB * 0 +
                    (cell_of_row % c.NB), cell_of_row // c.NB), nfetch_row)
    mx = int(cnt.max())
    return (order, qs, bs, js, ls, ss, row_grp, rank, nfetch_row,
            cell_of_row, cnt, mx)


def _fill_streams(cfg, parts):
    c = cfg
    (order, qs, bs, js, ls, ss, row_grp, rank, nfetch_row, cell_of_row,
     cnt, mx) = parts
    cell_off = np.zeros((c.NB, c.NCH), np.int64)
    off = 0
    for (b0, nblk) in c.GROUPS:
        for q in range(c.NCH):
            for br in range(nblk):
                cell_off[b0 + br, q] = off + br * c.CAP
            off += nblk * c.CAP
    assert off == c.TOTAL
    # fetch-instance position: cell_off + (running fetches of earlier rows
    # in this cell) + rank
    row_base = np.zeros(len(nfetch_row), np.int64)
    # running sum of nfetch_row within each cell (rows are cell-contiguous
    # after the lexsort since cell is the outer key)
    csum = np.cumsum(nfetch_row) - nfetch_row
    cell_first_row = np.zeros(len(nfetch_row), np.int64)
    # first row index of each cell group
    newcell = np.ones(len(nfetch_row), bool)
    newcell[1:] = cell_of_row[1:] != cell_of_row[:-1]
    cell_csum0 = np.where(newcell, csum, 0)
    cell_csum0 = np.maximum.accumulate(cell_csum0)
    row_base = csum - cell_csum0
    pos = cell_off[bs, qs] + row_base[row_grp] + rank
    idx_flat = np.zeros(c.TOTAL, np.int16)
    slA_flat = np.full(c.TOTAL, NOSLOT, np.float32)
    slB_flat = np.full(c.TOTAL, NOSLOT, np.float32)
    idx_flat[pos] = js.astype(np.int16)
    isA = ls == 0
    slA_flat[pos[isA]] = ss[isA]
    slB_flat[pos[~isA]] = ss[~isA]
    idxw = idx_flat.reshape(c.TOTAL // 16, 16).T.copy()
    idx_arr = np.tile(idxw, (8, 1))
    bf = ml_dtypes.bfloat16
    slA_arr = slA_flat.reshape(c.TTOT, P).T.astype(bf).copy()
    slB_arr = slB_flat.reshape(c.TTOT, P).T.astype(bf).copy()
    return idx_arr, slA_arr, slB_arr


_NC_CACHE = {}


def _get_nc(key, builder):
    if key not in _NC_CACHE:
        _NC_CACHE[key] = builder()
    return _NC_CACHE[key]


def kernel(verts, edges, W0_1, W1_1, b1, W0_2, W1_2, b2):
    global LAST_EXEC_NS
    LAST_EXEC_NS = []
    N, IN_DIM = verts.shape
    E = edges.shape[0]
    NCORES = 8
    H1 = W0_1.shape[1]
    H2 = W0_2.shape[1]
    cfg = Cfg(N, E, NCORES, IN_DIM, H1, H2)

    verts = np.asarray(verts, np.float32)
    edges = np.asarray(edges)
    src = np.asarray(edges[:, 0], np.int64)
    dst = np.asarray(edges[:, 1], np.int64)
    bf = ml_dtypes.bfloat16

    deg = np.bincount(src, minlength=cfg.NPADCOL).astype(np.float32)
    dinv = np.where(deg > 0, 1.0 / np.sqrt(np.maximum(deg, 1.0)),
                    0.0).astype(np.float32)

    vt1g = np.zeros((IN_DIM, cfg.NPADCOL), np.float32)
    vt1g[:, :N] = (verts * dinv[:N, None]).T

    CAPT = 3 * P
    cores = []
    tbq_need = 1
    for ci in range(NCORES):
        lo = ci * cfg.OWN
        m = (dst >= lo) & (dst < lo + cfg.OWN)
        es, edl = src[m], dst[m] - lo
        q_e = es // cfg.CHUNK_REAL
        cnt_vq = np.zeros((cfg.OWN, cfg.NCH), np.int64)
        np.add.at(cnt_vq, (edl, q_e), 1)
        block_of, slot_of, _ = _balance(cnt_vq, cfg.NB)
        b_e = block_of[edl]
        s_e = slot_of[edl].astype(np.float32)
        loc = (es % cfg.CHUNK_REAL).astype(np.int64)
        rowmaps, lanemaps, perms = [], [], []
        for q in range(cfg.NCH):
            sel = q_e == q
            rm, lm, pm = _pair_chunk(cfg, b_e[sel], loc[sel], CAPT)
            rowmaps.append(rm)
            lanemaps.append(lm)
            perms.append(pm)
        j_e = np.empty(len(es), np.int64)
        l_e = np.empty(len(es), np.int64)
        for q in range(cfg.NCH):
            sel = q_e == q
            j_e[sel] = rowmaps[q][loc[sel]]
            l_e[sel] = lanemaps[q][loc[sel]]
        parts = _fetch_streams(cfg, b_e, q_e, j_e, l_e, s_e)
        tbq_need = max(tbq_need, -(-parts[-1] // P))
        node_pos = block_of * P + slot_of
        cores.append((lo, parts, perms, node_pos))
    cfg.set_tbq(max(tbq_need, 1))

    iota_rep = np.broadcast_to(
        np.arange(P, dtype=np.float32)[None, :, None],
        (P, P, cfg.GRP * cfg.T_BQ)).astype(bf).copy()

    def make_dense_inputs(xmat, lo, node_pos, kdim):
        vox = np.zeros((kdim + 1, cfg.NODES_PAD), bf)
        xo = xmat[lo:lo + cfg.OWN]
        vox[0:kdim, node_pos] = xo.T.astype(bf)
        vox[kdim, node_pos] = np.ones((cfg.OWN,), bf)
        return vox

    in1_maps = []
    streams = []
    for (lo, parts, perms, node_pos) in cores:
        idx_arr, slA_arr, slB_arr = _fill_streams(cfg, parts)
        streams.append((idx_arr, slA_arr, slB_arr))
        dn = np.zeros(cfg.NODES_PAD, np.float32)
        dn[node_pos] = dinv[lo:lo + cfg.OWN]
        dinv_n = dn.reshape(cfg.NB, P).T.copy()
        # per-core column-permuted, dinv-prescaled vt
        gperm = np.concatenate(
            [q * cfg.CHUNK_REAL + perms[q] for q in range(cfg.NCH)])
        vt_core = vt1g[:, gperm].astype(bf)
        in1_maps.append({
            "vt": vt_core,
            "vox": make_dense_inputs(verts, lo, node_pos, IN_DIM),
            "w_t": W1_1.astype(bf),
            "w_d": np.vstack([np.asarray(W0_1, np.float32),
                              np.asarray(b1, np.float32)[None, :]]).astype(bf),
            "dinv_n": dinv_n, "idx": idx_arr, "slotA": slA_arr,
            "slotB": slB_arr, "iota": iota_rep,
        })

    trace = os.environ.get("MESHCONV_TRACE", "") == "1"

    nc1 = _get_nc(("l1", cfg.T_BQ),
                  lambda: _build_layer1(cfg, IN_DIM, H1))
    r1 = run_bass_kernel_spmd(nc1, in1_maps, core_ids=list(range(NCORES)),
                              trace=trace)
    if trace and r1.exec_time_ns:
        LAST_EXEC_NS.append(r1.exec_time_ns)

    # assemble full h [NPADCOL, H1] (un-permute block layout)
    h = np.zeros((cfg.NPADCOL, H1), np.float32)
    for ci, (lo, parts, perms, node_pos) in enumerate(cores):
        flat = np.asarray(r1.results[ci]["out"], dtype=np.float32)
        flat = flat.transpose(1, 0, 2).reshape(cfg.NODES_PAD, H1)
        h[lo:lo + cfg.OWN] = flat[node_pos]

    sh = (h * dinv[:, None]).astype(bf)               # [NPADCOL, H1]

    in2_maps = []
    for ci, (lo, parts, perms, node_pos) in enumerate(cores):
        idx_arr, slA_arr, slB_arr = streams[ci]
        # host-packed per-core lane tables: row (p,th) lanes from perm
        tbls = {}
        for q in range(cfg.NCH):
            a = sh[q * cfg.CHUNK_REAL + perms[q]]      # [CH, H1] pi order
            a = a.reshape(cfg.NT_CH, P, H1)            # [t, p, H1]
            t = np.zeros((P, cfg.NTH, P), bf)
            t[:, :, 0:H1] = a[0::2].transpose(1, 0, 2)
            t[:, :, 64:64 + H1] = a[1::2].transpose(1, 0, 2)
            tbls[f"table{q}"] = t
        dn = np.zeros(cfg.NODES_PAD, np.float32)
        dn[node_pos] = dinv[lo:lo + cfg.OWN]
        dinv_r = np.broadcast_to(dn[None, :].astype(bf),
                                 (H2, cfg.NODES_PAD)).copy()
        in2_maps.append({
            "vox": make_dense_inputs(h, lo, node_pos, H1),
            "w_t": W1_2.astype(bf),
            "w_d": np.vstack([np.asarray(W0_2, np.float32),
                              np.asarray(b2, np.float32)[None, :]]).astype(bf),
            "dinv_r": dinv_r, "idx": idx_arr, "slotA": slA_arr,
            "slotB": slB_arr, "iota": iota_rep, **tbls,
        })

    nc2 = _get_nc(("l2", cfg.T_BQ),
                  lambda: _build_layer2(cfg, H1, H2))
    r2 = run_bass_kernel_spmd(nc2, in2_maps, core_ids=list(range(NCORES)),
                              trace=trace)
    if trace and r2.exec_time_ns:
        LAST_EXEC_NS.append(r2.exec_time_ns)

    out = np.empty((N, H2), np.float32)
    for ci, (lo, parts, perms, node_pos) in enumerate(cores):
        flat = np.asarray(r2.results[ci]["out"], dtype=np.float32).T
        out[lo:lo + cfg.OWN] = flat[node_pos]
    return out
